# revision 10
# baseline (speedup 1.0000x reference)
"""Self-contained Trainium2 Bass kernel for the LSS voxel-pooling problem
(nn_DSFusionv2_28819230556604).

kernel(**inputs) takes the FULL unsharded inputs (numpy) and returns the
FULL [B, C, NZ, NY, NX] float32 output.

Strategy (8 NeuronCores, balanced scatter over frustum slices):
  The camera geometry makes voxel indices separable per (b,n,d) "slice":
  the x,y cell indices depend only on (n,d,w) and the z in-bounds mask
  depends only on (n,d,h).  The host mirrors the reference's float32 op
  sequence to get the indices, then:

    - keeps only in-bounds (b,n,d,h) feature rows (~88% of all rows),
    - packs the ~518 alive slices into 16 bins (2 chains x 8 cores) by
      kept-row count (FFD), so every core streams ~1/8 of the kept data,
    - packs each core's rows densely into [NG*128, FW*C] bf16.

  Device per core: stream the packed rows through the PE with a one-hot
  row->slice matmul (reduces over h), accumulating colsum[s, w, c] in
  PSUM.  Two independent accumulation chains (first/second half of the
  row groups) let the first chain's PSUM->SBUF copy and bf16 colsum
  store overlap the second chain's x stream.  No transpose, no second
  matmul stage.

  Host merges the per-slice column sums into the BEV canvas with one
  vectorized scatter-add (cell indices from the precomputed geometry).
"""
import os
import numpy as np
import ml_dtypes

# ---- problem constants (hardcoded from the reference config) ----
B, N, D, FH, FW, C = 2, 6, 48, 16, 44, 80
OGH, OGW = 256, 704
D_MIN, D_MAX = 2.0, 58.0
NX, NY, NZ = 256, 256, 1
LOWER = np.array([-51.2, -51.2, -10.0], np.float32)
DX = np.array([0.4, 0.4, 20.0], np.float32)

NCORE = 8
WC = FW * C                       # 3520


def _frustum():
    ds = D_MIN + (D_MAX - D_MIN) / D * np.arange(D, dtype=np.float32)
    ds = np.broadcast_to(ds[:, None, None], (D, FH, FW))
    xs = np.broadcast_to(np.linspace(0, OGW - 1, FW, dtype=np.float32)[None, None, :], (D, FH, FW))
    ys = np.broadcast_to(np.linspace(0, OGH - 1, FH, dtype=np.float32)[None, :, None], (D, FH, FW))
    return np.stack([xs, ys, ds], -1)


def _geometry_indices(rots, trans, intrins, post_rots, post_trans):
    """Voxel indices, bit-matching the reference's float32 op sequence."""
    frustum = _frustum()
    pts = frustum[None, None] - post_trans[:, :, None, None, None, :]
    inv_post = np.linalg.inv(post_rots).astype(np.float32)
    pts = np.einsum('bnij,bndhwj->bndhwi', inv_post, pts).astype(np.float32)
    pts = np.concatenate([pts[..., :2] * pts[..., 2:3], pts[..., 2:3]], axis=-1)
    combine = np.einsum('bnij,bnjk->bnik', rots,
                        np.linalg.inv(intrins).astype(np.float32)).astype(np.float32)
    pts = np.einsum('bnij,bndhwj->bndhwi', combine, pts).astype(np.float32)
    geom = (pts + trans[:, :, None, None, None, :]).astype(np.float32)
    gi = ((geom - LOWER) / DX).astype(np.int32)
    kept = ((gi[..., 0] >= 0) & (gi[..., 0] < NX) &
            (gi[..., 1] >= 0) & (gi[..., 1] < NY) &
            (gi[..., 2] >= 0) & (gi[..., 2] < NZ))
    return gi, kept


def _alive_slices(gi, kept):
    """Alive slices with kept h-rows and per-w cell indices."""
    slices = []
    for b in range(B):
        for n in range(N):
            for d in range(D):
                g = gi[b, n, d]
                k = kept[b, n, d]
                if not (g[..., 0] == g[0:1, :, 0]).all() or not (g[..., 1] == g[0:1, :, 1]).all():
                    raise RuntimeError("structure violation: gi_x/gi_y vary with h")
                zok = (g[:, :, 2] >= 0) & (g[:, :, 2] < NZ)
                if not (zok == zok[:, 0:1]).all():
                    raise RuntimeError("structure violation: z-ok varies with w")
                zh = zok[:, 0]
                xyok = ((g[0, :, 0] >= 0) & (g[0, :, 0] < NX) &
                        (g[0, :, 1] >= 0) & (g[0, :, 1] < NY))
                if not (k == (zh[:, None] & xyok[None, :])).all():
                    raise RuntimeError("structure violation: kept not separable")
                if not zh.any() or not xyok.any():
                    continue
                cells = np.where(xyok, g[0, :, 1].astype(np.int64) * NX + g[0, :, 0], -1)
                slices.append(dict(b=b, n=n, d=d, hs=np.nonzero(zh)[0], cells=cells))
    return slices


NCHAIN = 4


def _chain_groups(NG):
    """Split NG row-groups into NCHAIN contiguous chain ranges."""
    base, rem = divmod(NG, NCHAIN)
    gs = [0]
    for k in range(NCHAIN):
        gs.append(gs[-1] + base + (1 if k < rem else 0))
    return gs


def _pack_bins(slices):
    """FFD-pack slices into 32 bins (4 chains x 8 cores).

    Bin (k, c) holds chain k of core c: <= Gk*128 rows and <= 32 slices
    (a matmul PSUM output at partition offset 32k may span at most 32
    partitions)."""
    order = sorted(range(len(slices)), key=lambda i: -len(slices[i]["hs"]))
    total = sum(len(slices[i]["hs"]) for i in order)
    NG = max(NCHAIN, -(-total // (128 * NCORE)))
    while True:
        gs = _chain_groups(NG)
        caps = [(gs[k + 1] - gs[k]) * 128 for k in range(NCHAIN) for _ in range(NCORE)]
        load = [0] * (NCHAIN * NCORE)
        bins = [[] for _ in range(NCHAIN * NCORE)]
        ok = True
        for i in order:
            r = len(slices[i]["hs"])
            for j in range(NCHAIN * NCORE):
                if load[j] + r <= caps[j] and len(bins[j]) < 32:
                    bins[j].append(i)
                    load[j] += r
                    break
            else:
                ok = False
                break
        if ok:
            break
        NG += 1
    SK = [max(len(bins[k * NCORE + c]) for c in range(NCORE)) for k in range(NCHAIN)]
    return bins, NG, gs, SK


def _build_nc(NG, gs, SK):
    import concourse.bacc as bacc
    import concourse.mybir as mybir
    import concourse.tile as tile
    F32 = mybir.dt.float32
    BF16 = mybir.dt.bfloat16
    ST = sum(SK)
    OK = [sum(SK[:k]) for k in range(NCHAIN)]

    nc = bacc.Bacc(None, target_bir_lowering=True)
    x_d = nc.dram_tensor("x", [NG * 128, WC], BF16, kind="ExternalInput")
    z_d = nc.dram_tensor("z", [128, NG, 128], BF16, kind="ExternalInput")
    out_d = nc.dram_tensor("out", [ST, WC], BF16, kind="ExternalOutput")

    with tile.TileContext(nc) as tc:
        with (
            tc.tile_pool(name="sbuf", bufs=1) as pool,
            tc.tile_pool(name="xin", bufs=NG) as xpool,
            tc.tile_pool(name="psum", bufs=1, space="PSUM") as psum,
        ):
            Q = WC // 4
            ztile = pool.tile([128, NG, 128], BF16)
            obufs = [pool.tile([128, Q], BF16, name=f"obuf{q}") for q in range(4)]

            # issue every x stream DMA up front on the two HWDGE queues
            xgs = []
            for g in range(NG):
                xg = xpool.tile([128, WC], BF16)
                eng = nc.sync if g % 2 == 0 else nc.scalar
                if g == 1:
                    nc.scalar.dma_start(ztile[:], z_d[:])
                eng.dma_start(xg[:], x_d[128 * g:128 * (g + 1), :])
                xgs.append(xg)

            psumA = psum.tile([128, WC], F32, tag="ps")
            for g in range(NG):
                k = next(k for k in range(NCHAIN) if gs[k] <= g < gs[k + 1])
                p0 = 32 * k
                for o in range(0, WC, 512):
                    w = min(512, WC - o)
                    nc.tensor.matmul(
                        psumA[p0:p0 + SK[k], o:o + w],
                        ztile[:, g, p0:p0 + SK[k]], xgs[g][:, o:o + w],
                        start=(g == gs[k]), stop=(g == gs[k + 1] - 1),
                        skip_group_check=True,
                        tile_position=(0, p0),
                    )
                if g == gs[k + 1] - 1:
                    # chain k complete: drain its PSUM rows while the
                    # remaining chains keep streaming
                    for q in range(4):
                        sl = slice(q * Q, (q + 1) * Q)
                        if q % 2 == 0:
                            nc.vector.tensor_copy(obufs[q][p0:p0 + SK[k], :],
                                                  psumA[p0:p0 + SK[k], sl])
                        else:
                            nc.scalar.copy(obufs[q][p0:p0 + SK[k], :],
                                           psumA[p0:p0 + SK[k], sl])
                        nc.sync.dma_start(out_d[OK[k]:OK[k] + SK[k], sl],
                                          obufs[q][p0:p0 + SK[k], :])
    nc.compile()
    return nc


_NC_CACHE = {}
_LAST_EXEC_NS = None


def kernel(x, rots, trans, intrins, post_rots, post_trans):
    global _LAST_EXEC_NS
    x = np.asarray(x)
    rots = np.asarray(rots, np.float32)
    trans = np.asarray(trans, np.float32)
    intrins = np.asarray(intrins, np.float32)
    post_rots = np.asarray(post_rots, np.float32)
    post_trans = np.asarray(post_trans, np.float32)

    gi, kept = _geometry_indices(rots, trans, intrins, post_rots, post_trans)
    slices = _alive_slices(gi, kept)
    bins, NG, gs, SK = _pack_bins(slices)
    ST = sum(SK)
    OK = [sum(SK[:k]) for k in range(NCHAIN)]

    xb = x.astype(ml_dtypes.bfloat16)
    inmaps = []
    for c in range(NCORE):
        xc = np.zeros((NG * 128, WC), ml_dtypes.bfloat16)
        z = np.zeros((128, NG, 128), np.float32)
        for k in range(NCHAIN):
            rows_b, rows_n, rows_d, rows_h, rows_s = [], [], [], [], []
            for ls, i in enumerate(bins[k * NCORE + c]):
                sl = slices[i]
                for h in sl["hs"]:
                    rows_b.append(sl["b"]); rows_n.append(sl["n"])
                    rows_d.append(sl["d"]); rows_h.append(h)
                    rows_s.append(32 * k + ls)
            R = len(rows_s)
            r0 = gs[k] * 128
            xc[r0:r0 + R] = xb[rows_b, rows_n, rows_d, rows_h].reshape(R, WC)
            rr = np.arange(R)
            z[(r0 + rr) % 128, (r0 + rr) // 128, rows_s] = 1.0
        inmaps.append({"x": xc, "z": z.astype(ml_dtypes.bfloat16)})

    key = (NG, tuple(gs), tuple(SK))
    if key not in _NC_CACHE:
        _NC_CACHE[key] = _build_nc(NG, gs, SK)
    from concourse.bass_utils import run_bass_kernel_spmd
    trace = bool(int(os.environ.get("LSS_TRACE", "0")))
    if not trace:
        # the NTFF trace path needs antenv.axon_hooks, absent in this image;
        # make sure a global BASS_TRACE=1 can't route us there
        os.environ["BASS_NEVER_TRACE"] = "1"
    res = run_bass_kernel_spmd(_NC_CACHE[key], inmaps, core_ids=list(range(NCORE)),
                               trace=trace)
    _LAST_EXEC_NS = res.exec_time_ns

    # host merge: per-slice column sums -> BEV canvas
    canvas = np.zeros((B, NY * NX, C), np.float64)
    for c, r in zip(range(NCORE), res.results):
        dev = np.asarray(r["out"]).astype(np.float64).reshape(ST, FW, C)
        for k in range(NCHAIN):
            for ls, i in enumerate(bins[k * NCORE + c]):
                sl = slices[i]
                m = sl["cells"] >= 0
                np.add.at(canvas[sl["b"]], sl["cells"][m], dev[OK[k] + ls][m])
    out = (canvas.reshape(B, NY, NX, C).transpose(0, 3, 1, 2)[:, :, None]
           .astype(np.float32))
    return np.ascontiguousarray(out.reshape(B, C, NZ, NY, NX))


# revision 11
# speedup vs baseline: 1.3321x; 1.3321x over previous
"""Self-contained Trainium2 Bass kernel for the LSS voxel-pooling problem
(nn_DSFusionv2_28819230556604).

kernel(**inputs) takes the FULL unsharded inputs (numpy) and returns the
FULL [B, C, NZ, NY, NX] float32 output.

Strategy (8 NeuronCores, balanced scatter over frustum slices):
  The camera geometry makes voxel indices separable per (b,n,d) "slice":
  the x,y cell indices depend only on (n,d,w) and the z in-bounds mask
  depends only on (n,d,h).  The host mirrors the reference's float32 op
  sequence to get the indices, then:

    - keeps only in-bounds (b,n,d,h) feature rows (~88% of all rows),
    - load-balances the ~518 alive slices across the 8 cores by kept-row
      count (LPT), so every core streams ~1/8 of the kept data,
    - packs each core's rows densely into [NG*128, FW*C] bf16.

  Device per core: stream the packed rows through the PE with a one-hot
  row->slice matmul (reduces over h), accumulating colsum[s, w, c] in
  PSUM; drain PSUM to SBUF per 512-column bank chunk (vector/scalar
  engines, interleaved with the final group's matmuls) and DMA the bf16
  colsum out.  No transpose, no second matmul stage.

  Host merges the per-slice column sums into the BEV canvas with one
  vectorized scatter-add (cell indices from the precomputed geometry).
"""
import os
import numpy as np
import ml_dtypes

# ---- problem constants (hardcoded from the reference config) ----
B, N, D, FH, FW, C = 2, 6, 48, 16, 44, 80
OGH, OGW = 256, 704
D_MIN, D_MAX = 2.0, 58.0
NX, NY, NZ = 256, 256, 1
LOWER = np.array([-51.2, -51.2, -10.0], np.float32)
DX = np.array([0.4, 0.4, 20.0], np.float32)

NCORE = 8
WC = FW * C                       # 3520
CHUNKS = [(o, min(512, WC - o)) for o in range(0, WC, 512)]


def _frustum():
    ds = D_MIN + (D_MAX - D_MIN) / D * np.arange(D, dtype=np.float32)
    ds = np.broadcast_to(ds[:, None, None], (D, FH, FW))
    xs = np.broadcast_to(np.linspace(0, OGW - 1, FW, dtype=np.float32)[None, None, :], (D, FH, FW))
    ys = np.broadcast_to(np.linspace(0, OGH - 1, FH, dtype=np.float32)[None, :, None], (D, FH, FW))
    return np.stack([xs, ys, ds], -1)


def _geometry_indices(rots, trans, intrins, post_rots, post_trans):
    """Voxel indices, bit-matching the reference's float32 op sequence."""
    frustum = _frustum()
    pts = frustum[None, None] - post_trans[:, :, None, None, None, :]
    inv_post = np.linalg.inv(post_rots).astype(np.float32)
    pts = np.einsum('bnij,bndhwj->bndhwi', inv_post, pts).astype(np.float32)
    pts = np.concatenate([pts[..., :2] * pts[..., 2:3], pts[..., 2:3]], axis=-1)
    combine = np.einsum('bnij,bnjk->bnik', rots,
                        np.linalg.inv(intrins).astype(np.float32)).astype(np.float32)
    pts = np.einsum('bnij,bndhwj->bndhwi', combine, pts).astype(np.float32)
    geom = (pts + trans[:, :, None, None, None, :]).astype(np.float32)
    gi = ((geom - LOWER) / DX).astype(np.int32)
    kept = ((gi[..., 0] >= 0) & (gi[..., 0] < NX) &
            (gi[..., 1] >= 0) & (gi[..., 1] < NY) &
            (gi[..., 2] >= 0) & (gi[..., 2] < NZ))
    return gi, kept


def _alive_slices(gi, kept):
    """Alive slices with kept h-rows and per-w cell indices."""
    slices = []
    for b in range(B):
        for n in range(N):
            for d in range(D):
                g = gi[b, n, d]
                k = kept[b, n, d]
                if not (g[..., 0] == g[0:1, :, 0]).all() or not (g[..., 1] == g[0:1, :, 1]).all():
                    raise RuntimeError("structure violation: gi_x/gi_y vary with h")
                zok = (g[:, :, 2] >= 0) & (g[:, :, 2] < NZ)
                if not (zok == zok[:, 0:1]).all():
                    raise RuntimeError("structure violation: z-ok varies with w")
                zh = zok[:, 0]
                xyok = ((g[0, :, 0] >= 0) & (g[0, :, 0] < NX) &
                        (g[0, :, 1] >= 0) & (g[0, :, 1] < NY))
                if not (k == (zh[:, None] & xyok[None, :])).all():
                    raise RuntimeError("structure violation: kept not separable")
                if not zh.any() or not xyok.any():
                    continue
                cells = np.where(xyok, g[0, :, 1].astype(np.int64) * NX + g[0, :, 0], -1)
                slices.append(dict(b=b, n=n, d=d, hs=np.nonzero(zh)[0], cells=cells))
    return slices


def _pack_bins(slices):
    """LPT-pack slices into 8 per-core bins by kept-row count."""
    order = sorted(range(len(slices)), key=lambda i: -len(slices[i]["hs"]))
    load = [0] * NCORE
    bins = [[] for _ in range(NCORE)]
    for i in order:
        c = min(range(NCORE), key=lambda j: (load[j], j))
        bins[c].append(i)
        load[c] += len(slices[i]["hs"])
    ST = max(len(b) for b in bins)
    NG = -(-max(load) // 128)
    return bins, NG, ST


def _build_nc(NG, ST):
    import concourse.bacc as bacc
    import concourse.mybir as mybir
    import concourse.tile as tile
    F32 = mybir.dt.float32
    BF16 = mybir.dt.bfloat16

    nc = bacc.Bacc(None, target_bir_lowering=True)
    x_d = nc.dram_tensor("x", [NG * 128, WC], BF16, kind="ExternalInput")
    z_d = nc.dram_tensor("z", [128, NG, ST], BF16, kind="ExternalInput")
    out_d = nc.dram_tensor("out", [ST, WC], BF16, kind="ExternalOutput")

    with tile.TileContext(nc) as tc:
        with (
            tc.tile_pool(name="sbuf", bufs=1) as pool,
            tc.tile_pool(name="xin", bufs=NG) as xpool,
            tc.tile_pool(name="psum", bufs=1, space="PSUM") as psum,
        ):
            ztile = pool.tile([128, NG, ST], BF16)
            obufs = [pool.tile([128, 512], BF16, name=f"obuf{i}")
                     for i in range(len(CHUNKS))]

            # issue every x stream DMA up front on the two HWDGE queues
            xgs = []
            for g in range(NG):
                xg = xpool.tile([128, WC], BF16)
                if g == 0:
                    nc.scalar.dma_start(ztile[:], z_d[:])
                eng = nc.sync if g % 2 == 0 else nc.scalar
                eng.dma_start(xg[:], x_d[128 * g:128 * (g + 1), :])
                xgs.append(xg)

            psumA = psum.tile([128, WC], F32, tag="ps")
            for g in range(NG):
                for i, (o, w) in enumerate(CHUNKS):
                    nc.tensor.matmul(
                        psumA[0:ST, o:o + w],
                        ztile[:, g, :], xgs[g][:, o:o + w],
                        start=(g == 0), stop=(g == NG - 1),
                        skip_group_check=True,
                    )
                    if g == NG - 1:
                        # chunk fully accumulated: drain it while the PE
                        # finishes the remaining chunks
                        if i % 2 == 0:
                            nc.vector.tensor_copy(obufs[i][0:ST, 0:w],
                                                  psumA[0:ST, o:o + w])
                        else:
                            nc.scalar.copy(obufs[i][0:ST, 0:w],
                                           psumA[0:ST, o:o + w])
                        nc.sync.dma_start(out_d[:, o:o + w], obufs[i][0:ST, 0:w])
    nc.compile()
    return nc


_NC_CACHE = {}
_LAST_EXEC_NS = None


def kernel(x, rots, trans, intrins, post_rots, post_trans):
    global _LAST_EXEC_NS
    x = np.asarray(x)
    rots = np.asarray(rots, np.float32)
    trans = np.asarray(trans, np.float32)
    intrins = np.asarray(intrins, np.float32)
    post_rots = np.asarray(post_rots, np.float32)
    post_trans = np.asarray(post_trans, np.float32)

    gi, kept = _geometry_indices(rots, trans, intrins, post_rots, post_trans)
    slices = _alive_slices(gi, kept)
    bins, NG, ST = _pack_bins(slices)

    xb = x.astype(ml_dtypes.bfloat16)
    inmaps = []
    for c in range(NCORE):
        rows_b, rows_n, rows_d, rows_h, rows_s = [], [], [], [], []
        for ls, i in enumerate(bins[c]):
            sl = slices[i]
            for h in sl["hs"]:
                rows_b.append(sl["b"]); rows_n.append(sl["n"])
                rows_d.append(sl["d"]); rows_h.append(h)
                rows_s.append(ls)
        R = len(rows_s)
        xc = np.zeros((NG * 128, WC), ml_dtypes.bfloat16)
        xc[:R] = xb[rows_b, rows_n, rows_d, rows_h].reshape(R, WC)
        z = np.zeros((128, NG, ST), np.float32)
        rr = np.arange(R)
        z[rr % 128, rr // 128, rows_s] = 1.0
        inmaps.append({"x": xc, "z": z.astype(ml_dtypes.bfloat16)})

    key = (NG, ST)
    if key not in _NC_CACHE:
        _NC_CACHE[key] = _build_nc(NG, ST)
    from concourse.bass_utils import run_bass_kernel_spmd
    trace = bool(int(os.environ.get("LSS_TRACE", "0")))
    if not trace:
        # the NTFF trace path needs antenv.axon_hooks, absent in this image;
        # make sure a global BASS_TRACE=1 can't route us there
        os.environ["BASS_NEVER_TRACE"] = "1"
    res = run_bass_kernel_spmd(_NC_CACHE[key], inmaps, core_ids=list(range(NCORE)),
                               trace=trace)
    _LAST_EXEC_NS = res.exec_time_ns

    # host merge: per-slice column sums -> BEV canvas
    canvas = np.zeros((B, NY * NX, C), np.float64)
    for c, r in zip(range(NCORE), res.results):
        dev = np.asarray(r["out"]).astype(np.float64).reshape(ST, FW, C)
        for ls, i in enumerate(bins[c]):
            sl = slices[i]
            m = sl["cells"] >= 0
            np.add.at(canvas[sl["b"]], sl["cells"][m], dev[ls][m])
    out = (canvas.reshape(B, NY, NX, C).transpose(0, 3, 1, 2)[:, :, None]
           .astype(np.float32))
    return np.ascontiguousarray(out.reshape(B, C, NZ, NY, NX))
